# revision 10
# baseline (speedup 1.0000x reference)
"""Trainium2 Bass kernel for a dense transformer encoder layer.

Contract: kernel(**inputs) takes FULL unsharded inputs and returns the FULL
output [B, L, D] float32.

Sharding: 8 cores, data-parallel over batch (4) x sequence-split (2).
Core c handles batch b=c//2, sequence half h=c%2 (1024 query rows), computes
K/V over the full 2048 keys of its batch item (keys rotated so own rows come
first -> one identical SPMD program). No collectives.

v2 design (vs v0 baseline 533us / v1 488us):
  - fp8e4 DoubleRow matmuls (2x PE, measured) for QKV / V / out-proj / FFN2.
    Scores stay bf16 (K=64 DoubleRow measured 2x SLOWER). FFN1 stays bf16
    for accuracy (each fp8 FFN half costs ~1.3e-2 rel err; FFN2-fp8 chosen
    because FFN2 is the exposed tail). fp8 weights prescaled x16 into e4m3
    range; exp's ACT scale and scalar_tensor_tensor rescales fold them out.
  - AV ROW-major: psum [q,65], lhsT=exp tile, rhs=V[keys,65]; ~31ns/matmul
    (LDWEIGHTS hides under the 65-col stream). Softmax denominator lands on
    psum partitions: [128,1] reciprocal + tensor_scalar replaces v0's
    [1,512] reciprocal + gpsimd broadcast.
  - Fused front: V(t) matmuls issue right after tile t's LN transpose; kT
    pairs 0-1 right after lnT completes; kT 2-5 deferred into lch0
    attention.
  - Deferred-work scheduler: attention(lch) runs exp of head h+1 before AV
    of head h (head-level software pipeline), and between heads executes
    deferred closures -- lch0's out-proj/LN2 tiles and FFN1 chunks run
    inside lch1's attention so the PE works while ACT chews exp (~210us).
  - Tail: FFN2 runs as 2 passes of 4 psum accumulators (ps2 pool),
    interleaved with out-proj/LN2(lch1) tiles and FFN1(lch1) chunks.
  - Dedicated 1-bank psum pool for PE transposes so they never contend
    with matmul accumulators.
"""

import numpy as np
import ml_dtypes

B, L, D, H, I = 4, 2048, 768, 12, 3072
DH = D // H            # 64
P = 128
LQ = L // 2            # 1024 query rows per core
NCORES = 8
EPS = 1e-5

KD = D // P            # 6
KI = I // P            # 24
NT = L // P            # 16
NTQ = LQ // P          # 8
NPAIR = H // 2         # 6
VW = H * (DH + 1)      # 780

SQ = 16.0
SK = 16.0
SV = 16.0
SO = 16.0
SAO = 32.0             # aoT carries 32*ao (vaug ones col = 1/32)
S2 = 16.0              # W2 fp8 prescale (FFN2 fp8; FFN1 bf16)

_CACHE = {}


def _bf16(a):
    return np.ascontiguousarray(np.asarray(a, np.float32).astype(ml_dtypes.bfloat16))


def _f8(a):
    return np.ascontiguousarray(np.asarray(a, np.float32).astype(ml_dtypes.float8_e4m3))


def _f32(a):
    return np.ascontiguousarray(np.asarray(a, np.float32))


def _pm(vec, k):
    return np.ascontiguousarray(np.asarray(vec, np.float32).reshape(k, P).T)


def _wpm2(w):
    """[768, M] -> [128, 3, 2, M] DoubleRow k-pair layout (f = a*256+b*128+p)."""
    w = np.asarray(w)
    return np.ascontiguousarray(
        w.reshape(3, 2, P, w.shape[1]).transpose(2, 0, 1, 3))


def build(use_mask=False):
    import concourse.bass as bass
    import concourse.mybir as mybir
    import concourse.tile as tile
    from concourse import bacc
    from concourse.bass import ts
    from concourse.masks import make_identity
    from contextlib import ExitStack

    f32 = mybir.dt.float32
    bf16 = mybir.dt.bfloat16
    fp8 = mybir.dt.float8e4
    AF = mybir.ActivationFunctionType
    OP = mybir.AluOpType
    DR = mybir.MatmulPerfMode.DoubleRow

    nc = bacc.Bacc(None, target_bir_lowering=False, debug=False)

    x_d = nc.dram_tensor("xloc", [NT, P, D], f32, kind="ExternalInput")
    mb_d = nc.dram_tensor("mbias", [P, NT], f32, kind="ExternalInput")
    wqk_d = nc.dram_tensor("wqk", [P, KD, 2 * D], fp8, kind="ExternalInput")
    bqk_d = nc.dram_tensor("bqk", [P, 2 * KD], f32, kind="ExternalInput")
    wv_d = nc.dram_tensor("wv", [P, KD, D], fp8, kind="ExternalInput")
    bv_d = nc.dram_tensor("bv", [1, D], f32, kind="ExternalInput")
    wo_d = nc.dram_tensor("wo", [P, KD, D], fp8, kind="ExternalInput")
    bo_d = nc.dram_tensor("bo", [1, D], f32, kind="ExternalInput")
    w1_d = nc.dram_tensor("w1", [P, KD, I], bf16, kind="ExternalInput")
    b1_d = nc.dram_tensor("b1", [P, KI], f32, kind="ExternalInput")
    w2_d = nc.dram_tensor("w2", [P, KI, D], fp8, kind="ExternalInput")
    b2_d = nc.dram_tensor("b2", [1, D], f32, kind="ExternalInput")
    out_d = nc.dram_tensor("out", [NTQ, P, D], f32, kind="ExternalOutput")
    scr_d = nc.dram_tensor("warm_scr", [P, P], f32)

    wqk2 = wqk_d.rearrange("p (a b) m -> p a b m", b=2)
    wv2 = wv_d.rearrange("p (a b) m -> p a b m", b=2)
    wo2 = wo_d.rearrange("p (a b) m -> p a b m", b=2)
    w22 = w2_d.rearrange("p (a b) m -> p a b m", b=2)

    with ExitStack() as ctx:
        tc = ctx.enter_context(tile.TileContext(nc))
        ps = ctx.enter_context(tc.tile_pool(name="ps", bufs=2, space="PSUM"))
        ps2 = ctx.enter_context(tc.tile_pool(name="ps2", bufs=2, space="PSUM"))
        psT = ctx.enter_context(tc.tile_pool(name="psT", bufs=2, space="PSUM"))
        const = ctx.enter_context(tc.tile_pool(name="const", bufs=1))
        wres = ctx.enter_context(tc.tile_pool(name="wres", bufs=1))
        wstr = ctx.enter_context(tc.tile_pool(name="wstr", bufs=4))
        kvp = ctx.enter_context(tc.tile_pool(name="kvp", bufs=1))
        qkt = ctx.enter_context(tc.tile_pool(name="qkt", bufs=2))
        utp = ctx.enter_context(tc.tile_pool(name="utp", bufs=1))
        expp = ctx.enter_context(tc.tile_pool(name="expp", bufs=2))
        xp = ctx.enter_context(tc.tile_pool(name="xp", bufs=2))
        tp = ctx.enter_context(tc.tile_pool(name="tp", bufs=2))
        aop_p = ctx.enter_context(tc.tile_pool(name="aopp", bufs=2))

        nname = [0]

        def psum(cols=512):
            nname[0] += 1
            return ps.tile([P, cols], f32, tag="ps", name=f"ps{nname[0]}")

        def psum2():
            nname[0] += 1
            return ps2.tile([P, 2, 512], f32, tag="ps2", name=f"pp{nname[0]}")

        # ---- constants -----------------------------------------------------
        ident = const.tile([P, P], bf16, tag="ident")
        make_identity(nc, ident)
        epst = const.tile([P, 1], f32, tag="eps")
        nc.vector.memset(epst, EPS)
        mbias = const.tile([P, NT], f32, tag="mb")
        nc.sync.dma_start(mbias[:], mb_d[:])
        bqk_sb = const.tile([P, 2 * KD], f32, tag="bqk")
        nc.sync.dma_start(bqk_sb[:], bqk_d[:])
        bv_sb = const.tile([P, D], f32, tag="bv")
        nc.sync.dma_start(bv_sb[:], bv_d[:].to_broadcast((P, D)))
        bo_sb = const.tile([P, D], f32, tag="bo")
        nc.sync.dma_start(bo_sb[:], bo_d[:].to_broadcast((P, D)))
        b1_sb = const.tile([P, KI], f32, tag="b1")
        nc.sync.dma_start(b1_sb[:], b1_d[:])
        b2_sb = const.tile([P, D], f32, tag="b2")
        nc.sync.dma_start(b2_sb[:], b2_d[:].to_broadcast((P, D)))

        # persistent activations
        lnT = kvp.tile([P, KD, L], fp8, tag="lnu")
        vaug = kvp.tile([P, NT, VW], bf16, tag="vo")
        kTa = kvp.tile([P, NPAIR, L], bf16, tag="kT")
        aoT = kvp.tile([P, KD, LQ], fp8, tag="aoT")
        out1 = kvp.tile([P, NTQ, D], bf16, tag="o1")
        ln2T = kvp.tile([P, KD, LQ], bf16, tag="ln2T")

        lnT2 = lnT.rearrange("p (a b) l -> p a b l", b=2)
        aoT2 = aoT.rearrange("p (a b) l -> p a b l", b=2)

        exp_scale = float(1.0 / (SQ * SK * np.sqrt(DH)))

        def layernorm(dst, src, stats_tag):
            view = src.rearrange("p (a b) -> p a b", b=256)
            stats = tp.tile([P, 3, 6], f32, tag=stats_tag + "s")
            mv = tp.tile([P, 2], f32, tag=stats_tag + "m")
            for i in range(3):
                nc.vector.bn_stats(out=stats[:, i, :], in_=view[:, i, :])
            nc.vector.bn_aggr(out=mv[:], in_=stats[:])
            nc.scalar.activation(out=mv[:, 1:2], in_=mv[:, 1:2], func=AF.Sqrt,
                                 bias=epst[:], scale=1.0)
            nc.vector.reciprocal(out=mv[:, 1:2], in_=mv[:, 1:2])
            nc.vector.tensor_scalar(out=dst, in0=src,
                                    scalar1=mv[:, 0:1], scalar2=mv[:, 1:2],
                                    op0=OP.subtract, op1=OP.mult)

        gp_ok = [False]

        def transpose_128(dst, src_bf16, on_gp=False):
            nname[0] += 1
            pt = psT.tile([P, P], bf16, tag="psT", name=f"pt{nname[0]}")
            nc.tensor.transpose(pt[:], src_bf16, ident[:])
            if on_gp and gp_ok[0]:
                try:
                    nc.gpsimd.tensor_copy(out=dst, in_=pt[:])
                    return
                except Exception:
                    gp_ok[0] = False
            nc.vector.tensor_copy(out=dst, in_=pt[:])

        # ---- V for one key tile (fp8 DR) -----------------------------------
        wv_sb = wres.tile([P, KD // 2, 2, D], fp8, tag="wvw")
        nc.sync.dma_start(wv_sb[:], wv2[:])
        vview = vaug.rearrange("p t (h c) -> p t h c", c=DH + 1)
        nc.vector.memset(vview[:, :, :, DH : DH + 1], 1.0 / SAO)
        bv3 = bv_sb.rearrange("p (h c) -> p h c", c=DH)

        def emit_v(t):
            for ncol in range(2):
                pv = psum(384)
                for k in range(KD // 2):
                    nc.tensor.matmul(pv[:, :384], lnT2[:, k, :, ts(t, P)],
                                     wv_sb[:, k, :, ts(ncol, 384)],
                                     start=(k == 0), stop=(k == KD // 2 - 1),
                                     perf_mode=DR)
                dst = vview[:, t, 6 * ncol : 6 * ncol + 6, 0:DH]
                src = pv[:, :384].rearrange("p (h c) -> p h c", c=DH)
                bvb = bv3[:, 6 * ncol : 6 * ncol + 6, :]
                nc.vector.scalar_tensor_tensor(out=dst, in0=src, scalar=1.0 / SV,
                                               in1=bvb, op0=OP.mult, op1=OP.add)

        wqk_sb = wres.tile([P, KD // 2, 2, 2 * D], fp8, tag="wqkw")
        nc.sync.dma_start(wqk_sb[:], wqk2[:])

        # HAM warm-up first: PE clock ramps while DMAs land.
        wps = psum()
        for w in range(36):
            nc.tensor.matmul(wps[:, 0:P], ident[:], ident[:],
                             start=(w == 0), stop=(w == 35))
        wsb = tp.tile([P, P], f32, tag="wsb")
        nc.vector.tensor_copy(out=wsb[:], in_=wps[:, 0:P])
        nc.sync.dma_start(scr_d[:], wsb[:])

        # ---- Phase A: LN1 + transpose -> lnT, V(t) fused inline ------------
        for tpair in range(NT // 2):
            xt = xp.tile([P, 2, D], f32, tag="xl")
            if tpair == 0:
                nc.sync.dma_start(xt[:, 0, :], x_d[0])
                nc.sync.dma_start(xt[:, 1, :], x_d[1])
            else:
                nc.sync.dma_start(xt[:], x_d[2 * tpair : 2 * tpair + 2].rearrange("t p d -> p t d"))
            for s in range(2):
                t = 2 * tpair + s
                lnbf = tp.tile([P, D], bf16, tag="lnbf")
                layernorm(lnbf[:], xt[:, s, :], "ln1")
                for j in range(KD):
                    transpose_128(lnT[:, j, ts(t, P)], lnbf[:, ts(j, P)],
                                  on_gp=True)
                emit_v(t)

        def emit_kT(j):
            for nch in range(4):
                pk = psum()
                for k in range(KD // 2):
                    nc.tensor.matmul(pk[:], wqk_sb[:, k, :, D + j * P : D + (j + 1) * P],
                                     lnT2[:, k, :, ts(nch, 512)],
                                     start=(k == 0), stop=(k == KD // 2 - 1),
                                     perf_mode=DR)
                nc.vector.tensor_scalar(out=kTa[:, j, ts(nch, 512)], in0=pk[:],
                                        scalar1=bqk_sb[:, KD + j : KD + j + 1],
                                        scalar2=None, op0=OP.add)

        def emit_qT(lch, j):
            qTj = qkt.tile([P, 512], bf16, tag="qT", name=f"qT{lch}_{j}")
            pq = psum()
            for k in range(KD // 2):
                nc.tensor.matmul(pq[:], wqk_sb[:, k, :, ts(j, P)],
                                 lnT2[:, k, :, ts(lch, 512)],
                                 start=(k == 0), stop=(k == KD // 2 - 1),
                                 perf_mode=DR)
            nc.vector.tensor_scalar(out=qTj[:], in0=pq[:],
                                    scalar1=bqk_sb[:, j : j + 1], scalar2=None,
                                    op0=OP.add)
            return qTj

        def emit_scores_exp(lch, j, hh, qTj):
            r = hh * 64
            expT = expp.tile([P, NT, 512], bf16, tag="expT",
                             name=f"ex{lch}_{j}_{hh}")
            if use_mask:
                for mt in range(NT):
                    sc = psum()
                    nc.tensor.matmul(sc[:], kTa[r : r + 64, j, ts(mt, P)],
                                     qTj[r : r + 64, :], start=True, stop=True)
                    nc.scalar.activation(out=expT[:, mt, :], in_=sc[:],
                                         func=AF.Exp,
                                         bias=mbias[:, mt : mt + 1],
                                         scale=exp_scale)
            else:
                for mtp in range(NT // 2):
                    sc2 = psum2()
                    for s in range(2):
                        nc.tensor.matmul(sc2[:, s, :],
                                         kTa[r : r + 64, j, ts(2 * mtp + s, P)],
                                         qTj[r : r + 64, :], start=True, stop=True)
                    nc.scalar.activation(out=expT[:, 2 * mtp : 2 * mtp + 2, :],
                                         in_=sc2[:], func=AF.Exp, scale=exp_scale)
            return expT

        def emit_av_head(lch, j, hh, ex, aop):
            h = 2 * j + hh
            for qt in range(4):
                pav = psum(DH + 1)
                for mt in range(NT):
                    nc.tensor.matmul(pav[:], ex[:, mt, ts(qt, P)],
                                     vaug[:, mt, h * (DH + 1) : (h + 1) * (DH + 1)],
                                     start=(mt == 0), stop=(mt == NT - 1))
                rec = tp.tile([P, 1], f32, tag="rec")
                nc.vector.reciprocal(out=rec[:], in_=pav[:, DH : DH + 1])
                nc.vector.tensor_scalar(out=aop[:, qt, ts(hh, DH)],
                                        in0=pav[:, 0:DH],
                                        scalar1=rec[:], scalar2=None,
                                        op0=OP.mult)

        def emit_ao_transpose(lch, j, aop):
            for qt in range(4):
                transpose_128(
                    aoT[:, j, lch * 512 + qt * P : lch * 512 + (qt + 1) * P],
                    aop[:, qt, :])

        wo_sb = wres.tile([P, KD // 2, 2, D], fp8, tag="wow")
        nc.sync.dma_start(wo_sb[:], wo2[:])

        def emit_outproj_tile(lch, tt):
            t = lch * 4 + tt
            xr = xp.tile([P, D], f32, tag="xl")
            nc.sync.dma_start(xr[:], x_d[t].rearrange("p d -> p d"))
            for ncol in range(2):
                po = psum(384)
                for k in range(KD // 2):
                    nc.tensor.matmul(po[:, :384], aoT2[:, k, :, ts(t, P)],
                                     wo_sb[:, k, :, ts(ncol, 384)],
                                     start=(k == 0), stop=(k == KD // 2 - 1),
                                     perf_mode=DR)
                tmp = tp.tile([P, 384], f32, tag="zb")
                nc.vector.scalar_tensor_tensor(out=tmp[:], in0=po[:, :384],
                                               scalar=1.0 / (SAO * SO),
                                               in1=bo_sb[:, ts(ncol, 384)],
                                               op0=OP.mult, op1=OP.add)
                nc.vector.tensor_tensor(out=out1[:, t, ts(ncol, 384)],
                                        in0=tmp[:], in1=xr[:, ts(ncol, 384)],
                                        op=OP.add)

        def emit_ln2_tile(lch, tt):
            t = lch * 4 + tt
            lnbf = tp.tile([P, D], bf16, tag="lnbf")
            layernorm(lnbf[:], out1[:, t, :], "ln2")
            for k in range(KD):
                transpose_128(ln2T[:, k, ts(t, P)], lnbf[:, ts(k, P)])

        uT_t = [None, None]

        def emit_ffn1_chunk(lch, mts):
            uT = uT_t[lch]
            for mt in mts:
                w1t = wstr.tile([P, KD, P], bf16, tag="w1s")
                nc.sync.dma_start(w1t[:], w1_d[:, :, ts(mt, P)])
                pu = psum()
                for k in range(KD):
                    nc.tensor.matmul(pu[:], w1t[:, k, :],
                                     ln2T[:, k, ts(lch, 512)],
                                     start=(k == 0), stop=(k == KD - 1))
                nc.vector.tensor_scalar(out=uT[:, mt, :], in0=pu[:],
                                        scalar1=b1_sb[:, mt : mt + 1],
                                        scalar2=None, op0=OP.add)

        def emit_ffn2_pass(lch, half, pcb):
            """One pass: tt in {2*half, 2*half+1}; 4 accumulators in ps2.
            pcb: list of (after_kp, closure) run between kp chunks."""
            uT = uT_t[lch]
            uT2 = uT.rearrange("p (a b) n -> p a b n", b=2)
            pza, pzb = psum2(), psum2()
            pz = [[pza[:, 0, :384], pza[:, 1, :384]],
                  [pzb[:, 0, :384], pzb[:, 1, :384]]]
            ci = 0
            for kp in range(KI // 2):
                w2t = wstr.tile([P, 2, D], fp8, tag="w2s")
                nc.sync.dma_start(w2t[:], w22[:, kp, :, :])
                for i in range(2):
                    tt = 2 * half + i
                    for ncol in range(2):
                        nc.tensor.matmul(pz[i][ncol],
                                         uT2[:, kp, :, ts(tt, P)],
                                         w2t[:, :, ts(ncol, 384)],
                                         start=(kp == 0), stop=(kp == KI // 2 - 1),
                                         perf_mode=DR)
                while ci < len(pcb) and pcb[ci][0] == kp:
                    pcb[ci][1]()
                    ci += 1
            for i in range(2):
                tt = 2 * half + i
                t = lch * 4 + tt
                osb = tp.tile([P, D], f32, tag="osb")
                for ncol in range(2):
                    zb = tp.tile([P, 384], f32, tag="zb")
                    nc.vector.scalar_tensor_tensor(out=zb[:], in0=pz[i][ncol],
                                                   scalar=float(1.0 / S2),
                                                   in1=b2_sb[:, ts(ncol, 384)],
                                                   op0=OP.mult, op1=OP.add)
                    gt = tp.tile([P, 384], f32, tag="gt")
                    nc.scalar.activation(out=gt[:], in_=zb[:], func=AF.Gelu)
                    nc.vector.tensor_tensor(out=osb[:, ts(ncol, 384)], in0=gt[:],
                                            in1=out1[:, t, ts(ncol, 384)], op=OP.add)
                nc.sync.dma_start(out_d[t], osb[:])
            while ci < len(pcb):
                pcb[ci][1]()
                ci += 1

        # ---- schedule ------------------------------------------------------
        def attention(lch, deferred):
            d = 0
            pend = None
            aop_cur = [None]
            heads = [(j, hh) for j in range(NPAIR) for hh in range(2)]
            qT_hold = None
            for j, hh in heads:
                if hh == 0:
                    qT_hold = emit_qT(lch, j)
                    aop_cur[0] = aop_p.tile([P, 4, P], bf16, tag="aop",
                                            name=f"ao{lch}_{j}")
                ex = emit_scores_exp(lch, j, hh, qT_hold)
                aop = aop_cur[0]
                if pend is not None:
                    pj, phh, pex, paop = pend
                    emit_av_head(lch, pj, phh, pex, paop)
                    if phh == 1:
                        emit_ao_transpose(lch, pj, paop)
                if d < len(deferred):
                    deferred[d]()
                    d += 1
                pend = (j, hh, ex, aop)
            pj, phh, pex, paop = pend
            emit_av_head(lch, pj, phh, pex, paop)
            emit_ao_transpose(lch, pj, paop)
            while d < len(deferred):
                deferred[d]()
                d += 1

        emit_kT(0)
        emit_kT(1)
        attention(0, [lambda j=j: emit_kT(j) for j in range(2, NPAIR)])

        uT_t[0] = utp.tile([P, KI, 512], fp8, tag="uT", name="uT0")
        # lch1 attention: out-proj(0) tiles, batched LN2(0), FFN1(0) chunks
        deferred = [lambda tt=tt: emit_outproj_tile(0, tt) for tt in range(4)]
        deferred += [lambda: (emit_ln2_tile(0, 0), emit_ln2_tile(0, 1)),
                     lambda: (emit_ln2_tile(0, 2), emit_ln2_tile(0, 3))]
        deferred += [lambda mts=list(range(4 * c, 4 * c + 4)): emit_ffn1_chunk(0, mts)
                     for c in range(6)]
        attention(1, deferred)

        uT_t[1] = utp.tile([P, KI, 512], fp8, tag="uT", name="uT1")
        # tail: FFN2(0) two passes, interleaved with out-proj/LN2(1) tiles
        emit_ffn2_pass(0, 0, [(1, lambda: emit_outproj_tile(1, 0)),
                              (4, lambda: emit_outproj_tile(1, 1)),
                              (7, lambda: emit_outproj_tile(1, 2)),
                              (10, lambda: emit_outproj_tile(1, 3))])
        emit_ffn2_pass(0, 1, [(2, lambda: (emit_ln2_tile(1, 0), emit_ln2_tile(1, 1))),
                              (7, lambda: (emit_ln2_tile(1, 2), emit_ln2_tile(1, 3)))])
        # FFN1(1) chunks interleaved into FFN2(1) pass A; pass B last
        f1c = [lambda mts=list(range(3 * c, 3 * c + 3)): emit_ffn1_chunk(1, mts)
               for c in range(8)]
        f1c[0](); f1c[1](); f1c[2]()
        emit_ffn2_pass(1, 0, [(1, f1c[3]), (3, f1c[4]), (5, f1c[5]),
                              (7, f1c[6]), (9, f1c[7])])
        emit_ffn2_pass(1, 1, [])

    nc.compile()
    return nc


def _prep_host(x, attention_mask, ln1_g, ln1_b, Wqkv, bqkv, Wo, bo,
               ln2_g, ln2_b, W1, b1, W2, b2):
    x = _f32(x); mask = np.asarray(attention_mask)
    ln1_g = _f32(ln1_g); ln1_b = _f32(ln1_b)
    Wqkv = _f32(Wqkv); bqkv = _f32(bqkv)
    Wo = _f32(Wo); bo = _f32(bo)
    ln2_g = _f32(ln2_g); ln2_b = _f32(ln2_b)
    W1 = _f32(W1); b1 = _f32(b1); W2 = _f32(W2); b2 = _f32(b2)

    base = np.arange(H)[:, None] * 3 * DH
    q_idx = (base + np.arange(DH)).ravel()
    k_idx = (base + DH + np.arange(DH)).ravel()
    v_idx = (base + 2 * DH + np.arange(DH)).ravel()

    Wq = ln1_g[:, None] * Wqkv[:, q_idx] * SQ
    Wk = ln1_g[:, None] * Wqkv[:, k_idx] * SK
    Wv = ln1_g[:, None] * Wqkv[:, v_idx] * SV
    bq = (bqkv[q_idx] + ln1_b @ Wqkv[:, q_idx]) * SQ
    bk = (bqkv[k_idx] + ln1_b @ Wqkv[:, k_idx]) * SK
    bv = bqkv[v_idx] + ln1_b @ Wqkv[:, v_idx]
    W1p = ln2_g[:, None] * W1
    b1p = b1 + ln2_b @ W1

    shared = {
        "wqk": _f8(_wpm2(np.concatenate([Wq, Wk], axis=1)).reshape(P, KD, 2 * D)),
        "bqk": np.ascontiguousarray(
            np.concatenate([_pm(bq, KD), _pm(bk, KD)], axis=1)),
        "wv": _f8(_wpm2(Wv).reshape(P, KD, D)),
        "bv": _f32(bv[None, :]),
        "wo": _f8(_wpm2(Wo * SO).reshape(P, KD, D)),
        "bo": _f32(bo[None, :]),
        "w1": _bf16(np.ascontiguousarray(
            W1p.reshape(KD, P, I).transpose(1, 0, 2))),
        "b1": _pm(b1p, KI),
        "w2": _f8(np.ascontiguousarray(
            (W2 * S2).reshape(KI // 2, 2, P, D).transpose(2, 0, 1, 3)).reshape(P, KI, D)),
        "b2": _f32(b2[None, :]),
    }

    in_maps = []
    for c in range(NCORES):
        b, half = c // 2, c % 2
        own = slice(half * LQ, (half + 1) * LQ)
        oth = slice((1 - half) * LQ, (2 - half) * LQ)
        xl = np.concatenate([x[b, own], x[b, oth]], axis=0)
        ml = np.concatenate([mask[b, own], mask[b, oth]], axis=0)
        mb = (ml.astype(np.float32) - 1.0) * 30.0
        m = dict(shared)
        m["xloc"] = np.ascontiguousarray(xl.reshape(NT, P, D))
        m["mbias"] = np.ascontiguousarray(mb.reshape(NT, P).T)
        in_maps.append(m)
    return in_maps


LAST_RESULT = None
TRACE = False


def kernel(**inputs):
    global LAST_RESULT
    from concourse.bass_utils import run_bass_kernel_spmd

    use_mask = not bool(np.asarray(inputs["attention_mask"]).all())
    key = f"nc{int(use_mask)}"
    if key not in _CACHE:
        _CACHE[key] = build(use_mask)
    nc = _CACHE[key]

    in_maps = _prep_host(**inputs)
    res = run_bass_kernel_spmd(nc, in_maps, list(range(NCORES)), trace=TRACE)
    LAST_RESULT = res

    out = np.empty((B, L, D), np.float32)
    for c in range(NCORES):
        b, half = c // 2, c % 2
        o = res.results[c]["out"].reshape(LQ, D)
        out[b, half * LQ : (half + 1) * LQ] = o
    return out
